# revision 11
# baseline (speedup 1.0000x reference)
"""DDPM sampling kernel for Trainium2 (8 NeuronCores, Bass/Tile).

Strategy:
  - Factorized psi(state, action) computed once on-device (replicated).
  - 50 sequential reverse-diffusion steps, fully unrolled.
  - zeta MLP feature-major on-chip: L0 (128->1024), L1 (1024->1024),
    L2 (1024->32768) with the 32768-wide output projection column-sharded
    8 ways (32 FEAT-features per core, z_w2 shard SBUF-resident).
  - score = einsum('bf,bfs->bs', psi, zeta) folded into the L2 psum drain
    via scalar_tensor_tensor with per-partition psi scalars (batch-major).
  - Per-step AllReduce(score partial, 512KB) across the 8 cores.
  - Mish via rational identity mish(x) = x - x * 2/((1+e^x)^2+1)
    (ACT Exp+Square, DVE clamp/reciprocal/fused-mac) - no table switches.
  - fp32r (TF32-class) matmuls everywhere, N>=256 for full PE rate.

Layouts: activations feature-major [feat_part, B_free]; L2 psum batch-major
[b_part, j_free]; xt update feature-major after PE-transposing the
AllReduced score. Host pre-transposes inputs/outputs.
"""
import math
import numpy as np

import concourse.bass as bass
import concourse.mybir as mybir
import concourse.tile as tile
from concourse import bacc
from concourse.bass_utils import run_bass_kernel_spmd
from concourse.masks import make_identity

P = 128
B = 1024
T = 50
STATE = 128
ACT_D = 64
EMBED = 128
FEAT = 256
NCORES = 8
FLOC = FEAT // NCORES          # 32 features per core
JLOC = FLOC * STATE            # 4096 z_w2 columns per core
X_MIN, X_MAX = -1.0, 1.0
BETA_MIN, BETA_MAX = 1e-4, 0.02
MAX_POS = 10000.0

F32 = mybir.dt.float32
F32R = mybir.dt.float32r
AF = mybir.ActivationFunctionType
ALU = mybir.AluOpType

RSQ2 = float(1.0 / math.sqrt(2.0))
CLAMP = 3.0e38

_CACHE = {}


def _schedule():
    betas = np.linspace(BETA_MIN, BETA_MAX, T).astype(np.float64)
    alphas = 1.0 - betas
    abars = np.cumprod(alphas)
    abars_prev = np.concatenate([[1.0], abars[:-1]])
    # step i uses t = T-1-i
    c0c1 = []
    c1 = []
    sig = []
    for i in range(T):
        t = T - 1 - i
        beta = betas[t]
        ab = abars[t]
        abp = abars_prev[t]
        _c1 = 1.0 / math.sqrt(alphas[t])
        _c0 = beta / math.sqrt(1.0 - ab)
        s2 = max(beta * (1.0 - abp) / (1.0 - ab), 1e-20)
        _s = math.sqrt(s2) if t > 0 else 0.0
        c0c1.append(np.float32(_c0 * _c1))
        c1.append(np.float32(_c1))
        sig.append(np.float32(_s))
    return c0c1, c1, sig


def _pos_feat_T():
    # pos_feat for t-values in step order i -> t = T-1-i; returns [128, T]
    half = EMBED // 2
    freqs = (np.arange(half, dtype=np.float64) / half)
    freqs = (1.0 / MAX_POS) ** freqs
    ts = np.array([T - 1 - i for i in range(T)], dtype=np.float64)[:, None]
    x = ts @ freqs[None, :]                      # [T, half]
    pf = np.concatenate([np.cos(x), np.sin(x)], axis=1)   # [T, 128]
    return np.ascontiguousarray(pf.T.astype(np.float32))  # [128, T]


def _colchunks(v, nch):
    # bias vector [nch*128] -> [128, nch] (column k = bias[128k:128k+128])
    return np.ascontiguousarray(v.reshape(nch, P).T.astype(np.float32))


def _mish_drain(nc, sb, psum, bias_col, out_f32r_ap, wq, tag, rsq2=None):
    """mish(psum + bias) -> out (F32R). psum: [128, N] PSUM AP.
    wq: scratch pool. bias_col: [128,1] SBUF AP or 0.0."""
    n = psum.shape[-1]
    w = wq.tile([P, n], F32, tag=f"w{tag}", name=f"w_{tag}_{nc.next_id()}")
    # w = exp(x);  q' = ((w+1)/sqrt2)^2 = ((w+1)^2)/2
    nc.scalar.activation(w[:], psum, AF.Exp, bias=bias_col)
    q = wq.tile([P, n], F32, tag=f"q{tag}", name=f"q_{tag}_{nc.next_id()}")
    nc.scalar.activation(q[:], w[:], AF.Square, bias=rsq2, scale=rsq2)
    # q2h = clamp(q' + 0.5)  ( = ((1+e^x)^2+1)/2 );  r2 = 1/q2h = 2/(...)
    nc.vector.tensor_scalar(q[:], q[:], 0.5, CLAMP, ALU.add, ALU.min)
    r = wq.tile([P, n], F32, tag=f"r{tag}", name=f"r_{tag}_{nc.next_id()}")
    nc.vector.reciprocal_approx_fast(r[:], q[:])
    # u2 = (psum + bias) * r2 ;  out = (psum + bias) - u2
    u = wq.tile([P, n], F32, tag=f"u{tag}", name=f"u_{tag}_{nc.next_id()}")
    nc.vector.scalar_tensor_tensor(u[:], psum, bias_col, r[:], ALU.add, ALU.mult)
    nc.vector.scalar_tensor_tensor(out_f32r_ap, psum, bias_col, u[:],
                                   ALU.add, ALU.subtract)


def build(t_steps=T):
    nc = bacc.Bacc("TRN2", target_bir_lowering=False, debug=True)

    def din(name, shape):
        return nc.dram_tensor(name, list(shape), F32, kind="ExternalInput")

    # ---- inputs (feature-major where consumed as matmul rhs) ----
    stateT = din("stateT", [P, B])
    actionT = din("actionT", [P, B])           # padded rows 64:128 = 0
    xt0T = din("xt0T", [P, B])
    noiseT = din("noiseT", [T, P, B])
    s_w1 = din("s_w1", [P, 2 * EMBED]); s_b1 = din("s_b1", [P, 2])
    s_w2 = din("s_w2", [2 * EMBED, EMBED]); s_b2 = din("s_b2", [P, 1])
    a_w1 = din("a_w1", [P, 2 * EMBED]); a_b1 = din("a_b1", [P, 2])   # padded
    a_w2 = din("a_w2", [2 * EMBED, EMBED]); a_b2 = din("a_b2", [P, 1])
    t_w1 = din("t_w1", [P, 2 * EMBED]); t_b1 = din("t_b1", [P, 2])
    t_w2 = din("t_w2", [2 * EMBED, EMBED]); t_b2 = din("t_b2", [P, 1])
    p_w0 = din("p_w0", [2 * EMBED, 512]); p_b0 = din("p_b0", [P, 4])
    p_w1 = din("p_w1", [512, 512]); p_b1 = din("p_b1", [P, 4])
    p_w2s = din("p_w2s", [512, FLOC]); p_b2s = din("p_b2s", [FLOC, 1])
    z_w0a = din("z_w0a", [P, 1024])            # z_w0[:128, :]
    z_w0b = din("z_w0b", [P, 1024])            # z_w0[128:, :]
    z_b0 = din("z_b0", [P, 8])
    z_w1 = din("z_w1", [1024, 1024])
    z_b1 = din("z_b1", [P, 8])
    z_w2s = din("z_w2s", [1024, JLOC])         # per-core column shard
    z_b2sT = din("z_b2sT", [P, P])             # [f(32,pad128), s(128)]
    t_posT = din("t_posT", [P, T])
    out_xtT = nc.dram_tensor("out_xtT", [P, B], F32, kind="ExternalOutput")

    c0c1, c1v, sigv = _schedule()

    with tile.TileContext(nc) as tc:
        # ---------------- pools ----------------
        wpool = tc.alloc_tile_pool(name="wpool", bufs=1)
        ps = tc.alloc_tile_pool(name="ps", bufs=1, space="PSUM")
        _psctr = [0]

        def pstile(shape, name):
            _psctr[0] += 1
            return ps.tile(shape, F32, tag=f"psA{_psctr[0] % 4}", name=name)
        dram = tc.alloc_tile_pool(name="dram", bufs=2, space="DRAM")

        # resident constants (small); big weights allocated after setup frees
        zb1 = wpool.tile([P, 8], F32)
        nc.sync.dma_start(zb1[:], z_b1[:])
        ident = wpool.tile([P, P], F32)
        make_identity(nc, ident[:])
        c_rsq2 = wpool.tile([P, 1], F32)
        nc.vector.memset(c_rsq2[:], RSQ2)
        psi_bm = wpool.tile([P, 8, FLOC], F32)      # psi batch-major per btile
        sb_m = wpool.tile([P, 8, P], F32)           # score bias per btile
        b0T = wpool.tile([P, 8, T], F32)            # per-step L0 bias columns

        # ---------------- setup: psi, score-bias, t-embedding ----------------
        with tc.tile_pool(name="setup", bufs=1) as su:
            def ld(dr, shape, dtype=F32R, name=None):
                t_ = su.tile(list(shape), dtype, name=name or dr.name + "_t")
                if dtype == F32R:
                    nc.gpsimd.dma_start(t_[:], dr[:])
                else:
                    nc.sync.dma_start(t_[:], dr[:])
                return t_

            def ldc(dr, nch, f, name=None):
                # chunked weight [nch*128, f] -> [P, nch, f] (k%128 on partitions)
                t_ = su.tile([P, nch, f], F32R, name=name or dr.name + "_t")
                nc.gpsimd.dma_start(
                    t_[:], dr.ap().rearrange("(ko p) f -> p ko f", p=P))
                return t_

            sw1 = ld(s_w1, [P, 256]); aw1 = ld(a_w1, [P, 256])
            sw2 = ldc(s_w2, 2, P); aw2 = ldc(a_w2, 2, P)
            pw0 = ldc(p_w0, 2, 512); pw1 = ldc(p_w1, 4, 512)
            pw2 = ldc(p_w2s, 4, FLOC)
            tw1 = ld(t_w1, [P, 256]); tw2 = ldc(t_w2, 2, P)
            sb1 = ld(s_b1, [P, 2], F32); sb2 = ld(s_b2, [P, 1], F32)
            ab1 = ld(a_b1, [P, 2], F32); ab2 = ld(a_b2, [P, 1], F32)
            tb1 = ld(t_b1, [P, 2], F32); tb2 = ld(t_b2, [P, 1], F32)
            pb0 = ld(p_b0, [P, 4], F32); pb1 = ld(p_b1, [P, 4], F32)
            pb2 = ld(p_b2s, [FLOC, 1], F32)
            zb0 = ld(z_b0, [P, 8], F32)
            zw0b = ld(z_w0b, [P, 1024])
            zb2t = ld(z_b2sT, [P, P])
            stT = ld(stateT, [P, B]); acT = ld(actionT, [P, B])
            tpos = ld(t_posT, [P, T])

            # -- t embedding chain -> b0T --
            thm = su.tile([P, 2, T], F32R, name="thm")
            for fc in range(2):
                pt = pstile([P, T], f"su_th_{fc}")
                nc.tensor.matmul(pt[:], tw1[:, bass.ts(fc, P)], tpos[:],
                                 start=True, stop=True)
                _mish_drain(nc, su, pt[:], tb1[:, fc:fc + 1],
                            thm[:, fc, :], su, "su", c_rsq2[:])
            tff = su.tile([P, T], F32R, name="tff")
            pt = pstile([P, T], "su_tff")
            for k in range(2):
                nc.tensor.matmul(pt[:], tw2[:, k, :], thm[:, k, :],
                                 start=(k == 0), stop=(k == 1))
            nc.scalar.activation(tff[:], pt[:], AF.Identity, bias=tb2[:, 0:1])
            for fc in range(8):
                pt = pstile([P, T], f"su_b0_{fc}")
                nc.tensor.matmul(pt[:], zw0b[:, bass.ts(fc, P)], tff[:],
                                 start=True, stop=True)
                nc.scalar.activation(b0T[:, fc, :], pt[:], AF.Identity,
                                     bias=zb0[:, fc:fc + 1])

            # -- psi chain (feature-major, full B) --
            shm = su.tile([P, 2, B], F32R, name="shm")
            ahm = su.tile([P, 2, B], F32R, name="ahm")
            for fc in range(2):
                for h in range(2):
                    bsl = bass.ts(h, 512)
                    pt = pstile([P, 512], f"su_sh_{fc}_{h}")
                    nc.tensor.matmul(pt[:], sw1[:, bass.ts(fc, P)], stT[:, bsl],
                                     start=True, stop=True)
                    _mish_drain(nc, su, pt[:], sb1[:, fc:fc + 1],
                                shm[:, fc, bsl], su, "su", c_rsq2[:])
                    pt = pstile([P, 512], f"su_ah_{fc}_{h}")
                    nc.tensor.matmul(pt[:], aw1[:, bass.ts(fc, P)], acT[:, bsl],
                                     start=True, stop=True)
                    _mish_drain(nc, su, pt[:], ab1[:, fc:fc + 1],
                                ahm[:, fc, bsl], su, "su", c_rsq2[:])
            xp = su.tile([P, 2, B], F32R, name="xp")   # [s_ff ; a_ff]
            for h in range(2):
                bsl = bass.ts(h, 512)
                pt = pstile([P, 512], f"su_sff_{h}")
                for k in range(2):
                    nc.tensor.matmul(pt[:], sw2[:, k, :], shm[:, k, bsl],
                                     start=(k == 0), stop=(k == 1))
                nc.scalar.activation(xp[:, 0, bsl], pt[:], AF.Identity, bias=sb2[:, 0:1])
                pt = pstile([P, 512], f"su_aff_{h}")
                for k in range(2):
                    nc.tensor.matmul(pt[:], aw2[:, k, :], ahm[:, k, bsl],
                                     start=(k == 0), stop=(k == 1))
                nc.scalar.activation(xp[:, 1, bsl], pt[:], AF.Identity, bias=ab2[:, 0:1])
            phm = su.tile([P, 4, B], F32R, name="phm")
            for fc in range(4):
                for h in range(2):
                    bsl = bass.ts(h, 512)
                    pt = pstile([P, 512], f"su_ph_{fc}_{h}")
                    for k in range(2):
                        nc.tensor.matmul(pt[:], pw0[:, k, bass.ts(fc, P)],
                                         xp[:, k, bsl], start=(k == 0), stop=(k == 1))
                    _mish_drain(nc, su, pt[:], pb0[:, fc:fc + 1],
                                phm[:, fc, bsl], su, "su", c_rsq2[:])
            ph1m = su.tile([P, 4, B], F32R, name="ph1m")
            for fc in range(4):
                for h in range(2):
                    bsl = bass.ts(h, 512)
                    pt = pstile([P, 512], f"su_ph1_{fc}_{h}")
                    for k in range(4):
                        nc.tensor.matmul(pt[:], pw1[:, k, bass.ts(fc, P)],
                                         phm[:, k, bsl], start=(k == 0), stop=(k == 3))
                    _mish_drain(nc, su, pt[:], pb1[:, fc:fc + 1],
                                ph1m[:, fc, bsl], su, "su", c_rsq2[:])
            # psi_loc [32(pad128), B] fp32, padded rows zero
            psiT = su.tile([P, B], F32, name="psiT")
            nc.vector.memset(psiT[:], 0.0)
            for h in range(2):
                bsl = bass.ts(h, 512)
                pt = pstile([FLOC, 512], f"su_psi_{h}")
                for k in range(4):
                    nc.tensor.matmul(pt[:], pw2[:, k, :], ph1m[:, k, bsl],
                                     start=(k == 0), stop=(k == 3))
                nc.scalar.activation(psiT[:FLOC, bsl], pt[:], AF.Identity,
                                     bias=pb2[:, 0:1])
            psiTr = su.tile([P, B], F32R, name="psiTr")
            nc.gpsimd.tensor_copy(psiTr[:], psiT[:])
            # psi_bm via PE transpose; sb_m = psi_loc.T @ z_b2sT
            for m in range(8):
                msl = bass.ts(m, P)
                pt = pstile([P, P], f"su_tr_{m}")
                nc.tensor.transpose(pt[:], psiT[:, msl], ident[:])
                nc.scalar.activation(psi_bm[:, m, :], pt[:, :FLOC], AF.Copy)
                pt2 = pstile([P, P], f"su_sb_{m}")
                nc.tensor.matmul(pt2[:], psiTr[:, msl], zb2t[:],
                                 start=True, stop=True)
                nc.scalar.activation(sb_m[:, m, :], pt2[:], AF.Copy)

        # ---------------- resident weights (allocated post-setup) ----------
        wbig = tc.alloc_tile_pool(name="wbig", bufs=1)
        zw2 = wbig.tile([P, 8, JLOC], F32R)        # 128KB/part
        nc.gpsimd.dma_start(zw2[:], z_w2s.ap().rearrange("(ko p) j -> p ko j", p=P))
        zw0a = wbig.tile([P, 1024], F32R)
        nc.gpsimd.dma_start(zw0a[:], z_w0a[:])
        zw1r = z_w1.ap().rearrange("(ko p) f -> p ko f", p=P)

        # ---------------- main loop pools ----------------
        h0p = tc.alloc_tile_pool(name="h0p", bufs=1)
        h1p = tc.alloc_tile_pool(name="h1p", bufs=1)
        scr = tc.alloc_tile_pool(name="scr", bufs=2)
        xtp = tc.alloc_tile_pool(name="xtp", bufs=2)
        nzp = tc.alloc_tile_pool(name="nzp", bufs=1)
        scp = tc.alloc_tile_pool(name="scp", bufs=1)
        scorep = tc.alloc_tile_pool(name="scorep", bufs=2)
        zw1p = tc.alloc_tile_pool(name="zw1p", bufs=2)

        xtT = xtp.tile([P, B], F32R, name="xt_init")
        nc.gpsimd.dma_start(xtT[:], xt0T[:])

        NQ = 4
        QB = B // NQ   # 256

        for i in range(t_steps):
            nzT = nzp.tile([P, B], F32, tag="nz", name=f"nz_{i}")
            nc.sync.dma_start(nzT[:], noiseT[i])
            score_m = scorep.tile([P, 8, P], F32, tag="score", name=f"score_{i}")
            nc.vector.tensor_copy(score_m[:], sb_m[:])

            h0m_q = []
            h1m_q = []
            for q in range(NQ):
                qsl = bass.ts(q, QB)
                # ---- L0: h0 = mish(xt @ z_w0a + b0_t) ----
                h0m = h0p.tile([P, 8, QB], F32R, tag="h0", name=f"h0_{i}_{q}")
                h0m_q.append(h0m)
                for fc in range(8):
                    pt = ps.tile([P, QB], F32, tag=f"ps{'AB'[fc // 4]}{fc % 4}",
                                 name=f"l0_{i}_{q}_{fc}")
                    nc.tensor.matmul(pt[:], zw0a[:, bass.ts(fc, P)], xtT[:, qsl],
                                     start=True, stop=True)
                    _mish_drain(nc, scr, pt[:],
                                b0T[:, fc, i:i + 1], h0m[:, fc, :], scr, "m", c_rsq2[:])
                # ---- L1 (k-outer, z_w1 streamed, all 8 psum banks) ----
                h1m = h1p.tile([P, 8, QB], F32R, tag="h1", name=f"h1_{i}_{q}")
                h1m_q.append(h1m)
                l1ps = [ps.tile([P, QB], F32,
                                tag=f"ps{'AB'[fc // 4]}{fc % 4}",
                                name=f"l1_{i}_{q}_{fc}") for fc in range(8)]
                for k in range(8):
                    zw1t = zw1p.tile([P, 1024], F32R, tag="zw1",
                                     name=f"zw1_{i}_{q}_{k}")
                    nc.gpsimd.dma_start(zw1t[:], zw1r[:, k, :])
                    for fc in range(8):
                        nc.tensor.matmul(l1ps[fc][:], zw1t[:, bass.ts(fc, P)],
                                         h0m[:, k, :], start=(k == 0), stop=(k == 7))
                for fc in range(8):
                    _mish_drain(nc, scr, l1ps[fc][:],
                                zb1[:, fc:fc + 1], h1m[:, fc, :], scr, "m", c_rsq2[:])
                # ---- L2 + score ----
                for mb in range(2):
                    m = q * 2 + mb
                    for jt in range(8):
                        pt = ps.tile([P, 512], F32, tag=f"ps{'AB'[jt // 4]}{jt % 4}",
                                     name=f"l2_{i}_{m}_{jt}")
                        for k in range(8):
                            nc.tensor.matmul(
                                pt[:], h1m[:, k, bass.ts(mb, P)],
                                zw2[:, k, bass.ts(jt, 512)],
                                start=(k == 0), stop=(k == 7))
                        for sub in range(4):
                            f = 4 * jt + sub
                            nc.vector.scalar_tensor_tensor(
                                score_m[:, m, :], pt[:, bass.ts(sub, P)],
                                psi_bm[:, m, f:f + 1], score_m[:, m, :],
                                ALU.mult, ALU.add)

            # ---- AllReduce score partials ----
            arin = dram.tile([P, 8, P], F32, tag="arin", name=f"arin_{i}")
            arout = dram.tile([P, 8, P], F32, tag="arout", name=f"arout_{i}")
            nc.sync.dma_start(arin[:], score_m[:])
            nc.gpsimd.collective_compute(
                "AllReduce", ALU.add,
                replica_groups=[list(range(NCORES))],
                ins=[arin[:].opt()], outs=[arout[:].opt()])
            ar_rd = scp.tile([P, 8, P], F32, tag="arrd", name=f"arrd_{i}")
            nc.sync.dma_start(ar_rd[:], arout[:])

            # ---- update (feature-major): xt' = clip(c1*xt - c0c1*score + sig*z)
            upd = scp.tile([P, B], F32, tag="upd", name=f"upd_{i}")
            nc.vector.tensor_scalar_mul(upd[:], xtT[:].bitcast(F32), float(c1v[i]))
            for m in range(8):
                pt = ps.tile([P, P], F32, tag=f"ps{'AB'[m // 4]}{m % 4}", name=f"tr_{i}_{m}")
                nc.tensor.transpose(pt[:], ar_rd[:, m, :], ident[:])
                msl = bass.ts(m, P)
                nc.vector.scalar_tensor_tensor(
                    upd[:, msl], pt[:], -float(c0c1[i]), upd[:, msl],
                    ALU.mult, ALU.add)
            xtT = xtp.tile([P, B], F32R, tag="xt", name=f"xt_{i}")
            nc.vector.scalar_tensor_tensor(
                upd[:], nzT[:], float(sigv[i]), upd[:], ALU.mult, ALU.add)
            nc.vector.tensor_scalar(xtT[:], upd[:], X_MIN, X_MAX, ALU.max, ALU.min)

        nc.sync.dma_start(out_xtT[:], xtT[:].bitcast(F32))

        for p_ in (zw1p, scorep, scp, nzp, xtp, scr, h1p, h0p, dram, ps, wbig, wpool):
            p_.release()

    nc.compile()
    return nc


def _prep_inputs(state, action, xt_init, noise, params):
    p = params
    base = {
        "stateT": np.ascontiguousarray(state.T.astype(np.float32)),
        "actionT": np.ascontiguousarray(
            np.pad(action.T.astype(np.float32), ((0, P - ACT_D), (0, 0)))),
        "xt0T": np.ascontiguousarray(xt_init.T.astype(np.float32)),
        "noiseT": np.ascontiguousarray(noise.transpose(0, 2, 1).astype(np.float32)),
        "s_w1": np.asarray(p["s_w1"], np.float32), "s_b1": _colchunks(np.asarray(p["s_b1"]), 2),
        "s_w2": np.asarray(p["s_w2"], np.float32),
        "s_b2": _colchunks(np.asarray(p["s_b2"]), 1),
        "a_w1": np.pad(np.asarray(p["a_w1"], np.float32), ((0, P - ACT_D), (0, 0))),
        "a_b1": _colchunks(np.asarray(p["a_b1"]), 2),
        "a_w2": np.asarray(p["a_w2"], np.float32),
        "a_b2": _colchunks(np.asarray(p["a_b2"]), 1),
        "t_w1": np.asarray(p["t_w1"], np.float32), "t_b1": _colchunks(np.asarray(p["t_b1"]), 2),
        "t_w2": np.asarray(p["t_w2"], np.float32),
        "t_b2": _colchunks(np.asarray(p["t_b2"]), 1),
        "p_w0": np.asarray(p["p_w0"], np.float32),
        "p_b0": _colchunks(np.asarray(p["p_b0"]), 4),
        "p_w1": np.asarray(p["p_w1"], np.float32),
        "p_b1": _colchunks(np.asarray(p["p_b1"]), 4),
        "z_w0a": np.asarray(p["z_w0"], np.float32)[:P],
        "z_w0b": np.asarray(p["z_w0"], np.float32)[P:],
        "z_b0": _colchunks(np.asarray(p["z_b0"]), 8),
        "z_w1": np.asarray(p["z_w1"], np.float32),
        "z_b1": _colchunks(np.asarray(p["z_b1"]), 8),
        "t_posT": _pos_feat_T(),
    }
    z_w2 = np.asarray(p["z_w2"], np.float32)
    p_w2 = np.asarray(p["p_w2"], np.float32)
    p_b2 = np.asarray(p["p_b2"], np.float32)
    z_b2 = np.asarray(p["z_b2"], np.float32)
    in_maps = []
    for c in range(NCORES):
        m = dict(base)
        m["z_w2s"] = np.ascontiguousarray(z_w2[:, c * JLOC:(c + 1) * JLOC])
        m["p_w2s"] = np.ascontiguousarray(p_w2[:, c * FLOC:(c + 1) * FLOC])
        m["p_b2s"] = np.ascontiguousarray(
            p_b2[c * FLOC:(c + 1) * FLOC].reshape(FLOC, 1))
        zb2s = z_b2[c * JLOC:(c + 1) * JLOC].reshape(FLOC, STATE)
        m["z_b2sT"] = np.ascontiguousarray(np.pad(zb2s, ((0, P - FLOC), (0, 0))))
        in_maps.append(m)
    return in_maps


def kernel(state, action, xt_init, noise, params, _trace=False):
    if "nc" not in _CACHE:
        _CACHE["nc"] = build(T)
    nc = _CACHE["nc"]
    in_maps = _prep_inputs(state, action, xt_init, noise, params)
    r = run_bass_kernel_spmd(nc, in_maps, list(range(NCORES)), trace=_trace)
    _CACHE["last_result"] = r
    out = r.results[0]["out_xtT"]
    return np.ascontiguousarray(out.T.astype(np.float32))


# revision 13
# speedup vs baseline: 1.0395x; 1.0395x over previous
"""DDPM sampling kernel for Trainium2 (8 NeuronCores, Bass/Tile).

Strategy:
  - Factorized psi(state, action) computed once on-device (replicated).
  - 50 sequential reverse-diffusion steps, fully unrolled.
  - zeta MLP feature-major on-chip: L0 (128->1024), L1 (1024->1024),
    L2 (1024->32768) with the 32768-wide output projection column-sharded
    8 ways (32 FEAT-features per core, z_w2 shard SBUF-resident).
  - score = einsum('bf,bfs->bs', psi, zeta) folded into the L2 psum drain
    via scalar_tensor_tensor with per-partition psi scalars (batch-major).
  - Per-step AllReduce(score partial, 512KB) across the 8 cores.
  - Mish via rational identity mish(x) = x - x * 2/((1+e^x)^2+1)
    (ACT Exp+Square, DVE clamp/reciprocal/fused-mac) - no table switches.
  - fp32r (TF32-class) matmuls everywhere, N>=256 for full PE rate.

Layouts: activations feature-major [feat_part, B_free]; L2 psum batch-major
[b_part, j_free]; xt update feature-major after PE-transposing the
AllReduced score. Host pre-transposes inputs/outputs.
"""
import math
import numpy as np

import concourse.bass as bass
import concourse.mybir as mybir
import concourse.tile as tile
from concourse import bacc
from concourse.bass_utils import run_bass_kernel_spmd
from concourse.masks import make_identity

P = 128
B = 1024
T = 50
STATE = 128
ACT_D = 64
EMBED = 128
FEAT = 256
NCORES = 8
FLOC = FEAT // NCORES          # 32 features per core
JLOC = FLOC * STATE            # 4096 z_w2 columns per core
X_MIN, X_MAX = -1.0, 1.0
BETA_MIN, BETA_MAX = 1e-4, 0.02
MAX_POS = 10000.0

F32 = mybir.dt.float32
F32R = mybir.dt.float32r
AF = mybir.ActivationFunctionType
ALU = mybir.AluOpType

RSQ2 = float(1.0 / math.sqrt(2.0))
CLAMP = 3.0e38

_CACHE = {}


def _schedule():
    betas = np.linspace(BETA_MIN, BETA_MAX, T).astype(np.float64)
    alphas = 1.0 - betas
    abars = np.cumprod(alphas)
    abars_prev = np.concatenate([[1.0], abars[:-1]])
    # step i uses t = T-1-i
    c0c1 = []
    c1 = []
    sig = []
    for i in range(T):
        t = T - 1 - i
        beta = betas[t]
        ab = abars[t]
        abp = abars_prev[t]
        _c1 = 1.0 / math.sqrt(alphas[t])
        _c0 = beta / math.sqrt(1.0 - ab)
        s2 = max(beta * (1.0 - abp) / (1.0 - ab), 1e-20)
        _s = math.sqrt(s2) if t > 0 else 0.0
        c0c1.append(np.float32(_c0 * _c1))
        c1.append(np.float32(_c1))
        sig.append(np.float32(_s))
    return c0c1, c1, sig


def _pos_feat_T():
    # pos_feat for t-values in step order i -> t = T-1-i; returns [128, T]
    half = EMBED // 2
    freqs = (np.arange(half, dtype=np.float64) / half)
    freqs = (1.0 / MAX_POS) ** freqs
    ts = np.array([T - 1 - i for i in range(T)], dtype=np.float64)[:, None]
    x = ts @ freqs[None, :]                      # [T, half]
    pf = np.concatenate([np.cos(x), np.sin(x)], axis=1)   # [T, 128]
    return np.ascontiguousarray(pf.T.astype(np.float32))  # [128, T]


def _colchunks(v, nch):
    # bias vector [nch*128] -> [128, nch] (column k = bias[128k:128k+128])
    return np.ascontiguousarray(v.reshape(nch, P).T.astype(np.float32))


def _mish_drain(nc, sb, psum, bias_col, out_f32r_ap, wq, tag, rsq2=None):
    """mish(psum + bias) -> out (F32R). psum: [128, N] PSUM AP.
    wq: scratch pool. bias_col: [128,1] SBUF AP or 0.0."""
    n = psum.shape[-1]
    w = wq.tile([P, n], F32, tag=f"w{tag}", name=f"w_{tag}_{nc.next_id()}")
    # w = exp(x);  q' = ((w+1)/sqrt2)^2 = ((w+1)^2)/2
    nc.scalar.activation(w[:], psum, AF.Exp, bias=bias_col)
    q = wq.tile([P, n], F32, tag=f"q{tag}", name=f"q_{tag}_{nc.next_id()}")
    nc.scalar.activation(q[:], w[:], AF.Square, bias=rsq2, scale=rsq2)
    # q2h = clamp(q' + 0.5)  ( = ((1+e^x)^2+1)/2 );  r2 = 1/q2h = 2/(...)
    nc.vector.tensor_scalar(q[:], q[:], 0.5, CLAMP, ALU.add, ALU.min)
    r = wq.tile([P, n], F32, tag=f"r{tag}", name=f"r_{tag}_{nc.next_id()}")
    nc.vector.reciprocal_approx_fast(r[:], q[:])
    # u2 = (psum + bias) * r2 ;  out = (psum + bias) - u2
    u = wq.tile([P, n], F32, tag=f"u{tag}", name=f"u_{tag}_{nc.next_id()}")
    nc.vector.scalar_tensor_tensor(u[:], psum, bias_col, r[:], ALU.add, ALU.mult)
    nc.vector.scalar_tensor_tensor(out_f32r_ap, psum, bias_col, u[:],
                                   ALU.add, ALU.subtract)


def build(t_steps=T):
    nc = bacc.Bacc("TRN2", target_bir_lowering=False, debug=True)

    def din(name, shape):
        return nc.dram_tensor(name, list(shape), F32, kind="ExternalInput")

    # ---- inputs (feature-major where consumed as matmul rhs) ----
    stateT = din("stateT", [P, B])
    actionT = din("actionT", [P, B])           # padded rows 64:128 = 0
    xt0T = din("xt0T", [P, B])
    noiseT = din("noiseT", [T, P, B])
    s_w1 = din("s_w1", [P, 2 * EMBED]); s_b1 = din("s_b1", [P, 2])
    s_w2 = din("s_w2", [2 * EMBED, EMBED]); s_b2 = din("s_b2", [P, 1])
    a_w1 = din("a_w1", [P, 2 * EMBED]); a_b1 = din("a_b1", [P, 2])   # padded
    a_w2 = din("a_w2", [2 * EMBED, EMBED]); a_b2 = din("a_b2", [P, 1])
    t_w1 = din("t_w1", [P, 2 * EMBED]); t_b1 = din("t_b1", [P, 2])
    t_w2 = din("t_w2", [2 * EMBED, EMBED]); t_b2 = din("t_b2", [P, 1])
    p_w0 = din("p_w0", [2 * EMBED, 512]); p_b0 = din("p_b0", [P, 4])
    p_w1 = din("p_w1", [512, 512]); p_b1 = din("p_b1", [P, 4])
    p_w2s = din("p_w2s", [512, FLOC]); p_b2s = din("p_b2s", [FLOC, 1])
    z_w0a = din("z_w0a", [P, 1024])            # z_w0[:128, :]
    z_w0b = din("z_w0b", [P, 1024])            # z_w0[128:, :]
    z_b0 = din("z_b0", [P, 8])
    z_w1 = din("z_w1", [1024, 1024])
    z_b1 = din("z_b1", [P, 8])
    z_w2s = din("z_w2s", [1024, JLOC])         # per-core column shard
    z_b2sT = din("z_b2sT", [P, P])             # [f(32,pad128), s(128)]
    t_posT = din("t_posT", [P, T])
    out_xtT = nc.dram_tensor("out_xtT", [P, B], F32, kind="ExternalOutput")

    c0c1, c1v, sigv = _schedule()

    with tile.TileContext(nc) as tc:
        # ---------------- pools ----------------
        wpool = tc.alloc_tile_pool(name="wpool", bufs=1)
        ps = tc.alloc_tile_pool(name="ps", bufs=1, space="PSUM")
        _psctr = [0]

        def pstile(shape, name):
            _psctr[0] += 1
            return ps.tile(shape, F32, tag=f"psA{_psctr[0] % 4}", name=name)
        dram = tc.alloc_tile_pool(name="dram", bufs=2, space="DRAM")

        # resident constants (small); big weights allocated after setup frees
        zb1 = wpool.tile([P, 8], F32)
        nc.sync.dma_start(zb1[:], z_b1[:])
        ident = wpool.tile([P, P], F32)
        make_identity(nc, ident[:])
        c_rsq2 = wpool.tile([P, 1], F32)
        nc.vector.memset(c_rsq2[:], RSQ2)
        psi_bm = wpool.tile([P, 8, FLOC], F32)      # psi batch-major per btile
        sb_m = wpool.tile([P, 8, P], F32)           # score bias per btile
        b0T = wpool.tile([P, 8, T], F32)            # per-step L0 bias columns

        # ---------------- setup: psi, score-bias, t-embedding ----------------
        with tc.tile_pool(name="setup", bufs=1) as su:
            def ld(dr, shape, dtype=F32R, name=None):
                t_ = su.tile(list(shape), dtype, name=name or dr.name + "_t")
                if dtype == F32R:
                    nc.gpsimd.dma_start(t_[:], dr[:])
                else:
                    nc.sync.dma_start(t_[:], dr[:])
                return t_

            def ldc(dr, nch, f, name=None):
                # chunked weight [nch*128, f] -> [P, nch, f] (k%128 on partitions)
                t_ = su.tile([P, nch, f], F32R, name=name or dr.name + "_t")
                nc.gpsimd.dma_start(
                    t_[:], dr.ap().rearrange("(ko p) f -> p ko f", p=P))
                return t_

            sw1 = ld(s_w1, [P, 256]); aw1 = ld(a_w1, [P, 256])
            sw2 = ldc(s_w2, 2, P); aw2 = ldc(a_w2, 2, P)
            pw0 = ldc(p_w0, 2, 512); pw1 = ldc(p_w1, 4, 512)
            pw2 = ldc(p_w2s, 4, FLOC)
            tw1 = ld(t_w1, [P, 256]); tw2 = ldc(t_w2, 2, P)
            sb1 = ld(s_b1, [P, 2], F32); sb2 = ld(s_b2, [P, 1], F32)
            ab1 = ld(a_b1, [P, 2], F32); ab2 = ld(a_b2, [P, 1], F32)
            tb1 = ld(t_b1, [P, 2], F32); tb2 = ld(t_b2, [P, 1], F32)
            pb0 = ld(p_b0, [P, 4], F32); pb1 = ld(p_b1, [P, 4], F32)
            pb2 = ld(p_b2s, [FLOC, 1], F32)
            zb0 = ld(z_b0, [P, 8], F32)
            zw0b = ld(z_w0b, [P, 1024])
            zb2t = ld(z_b2sT, [P, P])
            stT = ld(stateT, [P, B]); acT = ld(actionT, [P, B])
            tpos = ld(t_posT, [P, T])

            # -- t embedding chain -> b0T --
            thm = su.tile([P, 2, T], F32R, name="thm")
            for fc in range(2):
                pt = pstile([P, T], f"su_th_{fc}")
                nc.tensor.matmul(pt[:], tw1[:, bass.ts(fc, P)], tpos[:],
                                 start=True, stop=True)
                _mish_drain(nc, su, pt[:], tb1[:, fc:fc + 1],
                            thm[:, fc, :], su, "su", c_rsq2[:])
            tff = su.tile([P, T], F32R, name="tff")
            pt = pstile([P, T], "su_tff")
            for k in range(2):
                nc.tensor.matmul(pt[:], tw2[:, k, :], thm[:, k, :],
                                 start=(k == 0), stop=(k == 1))
            nc.scalar.activation(tff[:], pt[:], AF.Identity, bias=tb2[:, 0:1])
            for fc in range(8):
                pt = pstile([P, T], f"su_b0_{fc}")
                nc.tensor.matmul(pt[:], zw0b[:, bass.ts(fc, P)], tff[:],
                                 start=True, stop=True)
                nc.scalar.activation(b0T[:, fc, :], pt[:], AF.Identity,
                                     bias=zb0[:, fc:fc + 1])

            # -- psi chain (feature-major, full B) --
            shm = su.tile([P, 2, B], F32R, name="shm")
            ahm = su.tile([P, 2, B], F32R, name="ahm")
            for fc in range(2):
                for h in range(2):
                    bsl = bass.ts(h, 512)
                    pt = pstile([P, 512], f"su_sh_{fc}_{h}")
                    nc.tensor.matmul(pt[:], sw1[:, bass.ts(fc, P)], stT[:, bsl],
                                     start=True, stop=True)
                    _mish_drain(nc, su, pt[:], sb1[:, fc:fc + 1],
                                shm[:, fc, bsl], su, "su", c_rsq2[:])
                    pt = pstile([P, 512], f"su_ah_{fc}_{h}")
                    nc.tensor.matmul(pt[:], aw1[:, bass.ts(fc, P)], acT[:, bsl],
                                     start=True, stop=True)
                    _mish_drain(nc, su, pt[:], ab1[:, fc:fc + 1],
                                ahm[:, fc, bsl], su, "su", c_rsq2[:])
            xp = su.tile([P, 2, B], F32R, name="xp")   # [s_ff ; a_ff]
            for h in range(2):
                bsl = bass.ts(h, 512)
                pt = pstile([P, 512], f"su_sff_{h}")
                for k in range(2):
                    nc.tensor.matmul(pt[:], sw2[:, k, :], shm[:, k, bsl],
                                     start=(k == 0), stop=(k == 1))
                nc.scalar.activation(xp[:, 0, bsl], pt[:], AF.Identity, bias=sb2[:, 0:1])
                pt = pstile([P, 512], f"su_aff_{h}")
                for k in range(2):
                    nc.tensor.matmul(pt[:], aw2[:, k, :], ahm[:, k, bsl],
                                     start=(k == 0), stop=(k == 1))
                nc.scalar.activation(xp[:, 1, bsl], pt[:], AF.Identity, bias=ab2[:, 0:1])
            phm = su.tile([P, 4, B], F32R, name="phm")
            for fc in range(4):
                for h in range(2):
                    bsl = bass.ts(h, 512)
                    pt = pstile([P, 512], f"su_ph_{fc}_{h}")
                    for k in range(2):
                        nc.tensor.matmul(pt[:], pw0[:, k, bass.ts(fc, P)],
                                         xp[:, k, bsl], start=(k == 0), stop=(k == 1))
                    _mish_drain(nc, su, pt[:], pb0[:, fc:fc + 1],
                                phm[:, fc, bsl], su, "su", c_rsq2[:])
            ph1m = su.tile([P, 4, B], F32R, name="ph1m")
            for fc in range(4):
                for h in range(2):
                    bsl = bass.ts(h, 512)
                    pt = pstile([P, 512], f"su_ph1_{fc}_{h}")
                    for k in range(4):
                        nc.tensor.matmul(pt[:], pw1[:, k, bass.ts(fc, P)],
                                         phm[:, k, bsl], start=(k == 0), stop=(k == 3))
                    _mish_drain(nc, su, pt[:], pb1[:, fc:fc + 1],
                                ph1m[:, fc, bsl], su, "su", c_rsq2[:])
            # psi_loc [32(pad128), B] fp32, padded rows zero
            psiT = su.tile([P, B], F32, name="psiT")
            nc.vector.memset(psiT[:], 0.0)
            for h in range(2):
                bsl = bass.ts(h, 512)
                pt = pstile([FLOC, 512], f"su_psi_{h}")
                for k in range(4):
                    nc.tensor.matmul(pt[:], pw2[:, k, :], ph1m[:, k, bsl],
                                     start=(k == 0), stop=(k == 3))
                nc.scalar.activation(psiT[:FLOC, bsl], pt[:], AF.Identity,
                                     bias=pb2[:, 0:1])
            psiTr = su.tile([P, B], F32R, name="psiTr")
            nc.gpsimd.tensor_copy(psiTr[:], psiT[:])
            # psi_bm via PE transpose; sb_m = psi_loc.T @ z_b2sT
            for m in range(8):
                msl = bass.ts(m, P)
                pt = pstile([P, P], f"su_tr_{m}")
                nc.tensor.transpose(pt[:], psiT[:, msl], ident[:])
                nc.scalar.activation(psi_bm[:, m, :], pt[:, :FLOC], AF.Copy)
                pt2 = pstile([P, P], f"su_sb_{m}")
                nc.tensor.matmul(pt2[:], psiTr[:, msl], zb2t[:],
                                 start=True, stop=True)
                nc.scalar.activation(sb_m[:, m, :], pt2[:], AF.Copy)

        # ---------------- resident weights (allocated post-setup) ----------
        wbig = tc.alloc_tile_pool(name="wbig", bufs=1)
        zw2 = wbig.tile([P, 8, JLOC], F32R)        # 128KB/part
        nc.gpsimd.dma_start(zw2[:], z_w2s.ap().rearrange("(ko p) j -> p ko j", p=P))
        zw0a = wbig.tile([P, 1024], F32R)
        nc.gpsimd.dma_start(zw0a[:], z_w0a[:])
        zw1r = z_w1.ap().rearrange("(ko p) f -> p ko f", p=P)

        # ---------------- main loop pools ----------------
        h0p = tc.alloc_tile_pool(name="h0p", bufs=1)
        h1p = tc.alloc_tile_pool(name="h1p", bufs=1)
        scr = tc.alloc_tile_pool(name="scr", bufs=2)
        xtp = tc.alloc_tile_pool(name="xtp", bufs=2)
        nzp = tc.alloc_tile_pool(name="nzp", bufs=1)
        scp = tc.alloc_tile_pool(name="scp", bufs=1)
        scorep = tc.alloc_tile_pool(name="scorep", bufs=1)
        zw1p = tc.alloc_tile_pool(name="zw1p", bufs=3)

        xtT = xtp.tile([P, B], F32R, name="xt_init")
        nc.gpsimd.dma_start(xtT[:], xt0T[:])

        NQ = 4
        QB = B // NQ   # 256

        for i in range(t_steps):
            nzT = nzp.tile([P, B], F32, tag="nz", name=f"nz_{i}")
            nc.sync.dma_start(nzT[:], noiseT[i])
            score_m = scorep.tile([P, 8, P], F32, tag="score", name=f"score_{i}")
            for hh in range(2):
                nc.vector.tensor_copy(score_m[:, 4 * hh:4 * hh + 4, :],
                                      sb_m[:, 4 * hh:4 * hh + 4, :])
            xt_new = xtp.tile([P, B], F32R, tag="xt", name=f"xt_{i}")

            h0m_q = []
            h1m_q = []
            for q in range(NQ):
                qsl = bass.ts(q, QB)
                # ---- L0: h0 = mish(xt @ z_w0a + b0_t) ----
                h0m = h0p.tile([P, 8, QB], F32R, tag="h0", name=f"h0_{i}_{q}")
                h0m_q.append(h0m)
                for fc in range(8):
                    pt = ps.tile([P, QB], F32, tag=f"ps{'AB'[fc // 4]}{fc % 4}",
                                 name=f"l0_{i}_{q}_{fc}")
                    nc.tensor.matmul(pt[:], zw0a[:, bass.ts(fc, P)], xtT[:, qsl],
                                     start=True, stop=True)
                    _mish_drain(nc, scr, pt[:],
                                b0T[:, fc, i:i + 1], h0m[:, fc, :], scr, "m", c_rsq2[:])
                # ---- L1 (k-outer, z_w1 streamed, all 8 psum banks) ----
                h1m = h1p.tile([P, 8, QB], F32R, tag="h1", name=f"h1_{i}_{q}")
                h1m_q.append(h1m)
                l1ps = [ps.tile([P, QB], F32,
                                tag=f"ps{'AB'[fc // 4]}{fc % 4}",
                                name=f"l1_{i}_{q}_{fc}") for fc in range(8)]
                for k in range(8):
                    zw1t = zw1p.tile([P, 1024], F32R, tag="zw1",
                                     name=f"zw1_{i}_{q}_{k}")
                    nc.gpsimd.dma_start(zw1t[:], zw1r[:, k, :])
                    for fc in range(8):
                        nc.tensor.matmul(l1ps[fc][:], zw1t[:, bass.ts(fc, P)],
                                         h0m[:, k, :], start=(k == 0), stop=(k == 7))
                for fc in range(8):
                    _mish_drain(nc, scr, l1ps[fc][:],
                                zb1[:, fc:fc + 1], h1m[:, fc, :], scr, "m", c_rsq2[:])
                # ---- L2 + score ----
                for mb in range(2):
                    m = q * 2 + mb
                    for jt in range(8):
                        pt = ps.tile([P, 512], F32, tag=f"ps{'AB'[jt // 4]}{jt % 4}",
                                     name=f"l2_{i}_{m}_{jt}")
                        for k in range(8):
                            nc.tensor.matmul(
                                pt[:], h1m[:, k, bass.ts(mb, P)],
                                zw2[:, k, bass.ts(jt, 512)],
                                start=(k == 0), stop=(k == 7))
                        for sub in range(4):
                            f = 4 * jt + sub
                            nc.vector.scalar_tensor_tensor(
                                score_m[:, m, :], pt[:, bass.ts(sub, P)],
                                psi_bm[:, m, f:f + 1], score_m[:, m, :],
                                ALU.mult, ALU.add)

                if q % 2 == 1:
                    # ---- half hh complete: AllReduce + update this half ----
                    hh = q // 2
                    hsl = bass.ds(hh * 512, 512)
                    arin = dram.tile([P, 4, P], F32, tag=f"arin{hh}",
                                     name=f"arin_{i}_{hh}")
                    arout = dram.tile([P, 4, P], F32, tag=f"arout{hh}",
                                      name=f"arout_{i}_{hh}")
                    nc.sync.dma_start(arin[:], score_m[:, 4 * hh:4 * hh + 4, :])
                    nc.gpsimd.collective_compute(
                        "AllReduce", ALU.add,
                        replica_groups=[list(range(NCORES))],
                        ins=[arin[:].opt()], outs=[arout[:].opt()])
                    ar_rd = scp.tile([P, 4, P], F32, tag=f"arrd{hh}",
                                     name=f"arrd_{i}_{hh}")
                    nc.sync.dma_start(ar_rd[:], arout[:])
                    upd = scp.tile([P, 512], F32, tag=f"upd{hh}",
                                   name=f"upd_{i}_{hh}")
                    nc.vector.tensor_scalar_mul(
                        upd[:], xtT[:, hsl].bitcast(F32), float(c1v[i]))
                    for mm_ in range(4):
                        m_g = 4 * hh + mm_
                        pt2 = ps.tile([P, P], F32,
                                      tag=f"ps{'AB'[m_g // 4]}{m_g % 4}",
                                      name=f"tr_{i}_{m_g}")
                        nc.tensor.transpose(pt2[:], ar_rd[:, mm_, :], ident[:])
                        msl = bass.ts(mm_, P)
                        nc.vector.scalar_tensor_tensor(
                            upd[:, msl], pt2[:], -float(c0c1[i]), upd[:, msl],
                            ALU.mult, ALU.add)
                    nc.vector.scalar_tensor_tensor(
                        upd[:], nzT[:, hsl], float(sigv[i]), upd[:],
                        ALU.mult, ALU.add)
                    nc.vector.tensor_scalar(xt_new[:, hsl], upd[:],
                                            X_MIN, X_MAX, ALU.max, ALU.min)

            xtT = xt_new

        nc.sync.dma_start(out_xtT[:], xtT[:].bitcast(F32))

        for p_ in (zw1p, scorep, scp, nzp, xtp, scr, h1p, h0p, dram, ps, wbig, wpool):
            p_.release()

    nc.compile()
    return nc


def _prep_inputs(state, action, xt_init, noise, params):
    p = params
    base = {
        "stateT": np.ascontiguousarray(state.T.astype(np.float32)),
        "actionT": np.ascontiguousarray(
            np.pad(action.T.astype(np.float32), ((0, P - ACT_D), (0, 0)))),
        "xt0T": np.ascontiguousarray(xt_init.T.astype(np.float32)),
        "noiseT": np.ascontiguousarray(noise.transpose(0, 2, 1).astype(np.float32)),
        "s_w1": np.asarray(p["s_w1"], np.float32), "s_b1": _colchunks(np.asarray(p["s_b1"]), 2),
        "s_w2": np.asarray(p["s_w2"], np.float32),
        "s_b2": _colchunks(np.asarray(p["s_b2"]), 1),
        "a_w1": np.pad(np.asarray(p["a_w1"], np.float32), ((0, P - ACT_D), (0, 0))),
        "a_b1": _colchunks(np.asarray(p["a_b1"]), 2),
        "a_w2": np.asarray(p["a_w2"], np.float32),
        "a_b2": _colchunks(np.asarray(p["a_b2"]), 1),
        "t_w1": np.asarray(p["t_w1"], np.float32), "t_b1": _colchunks(np.asarray(p["t_b1"]), 2),
        "t_w2": np.asarray(p["t_w2"], np.float32),
        "t_b2": _colchunks(np.asarray(p["t_b2"]), 1),
        "p_w0": np.asarray(p["p_w0"], np.float32),
        "p_b0": _colchunks(np.asarray(p["p_b0"]), 4),
        "p_w1": np.asarray(p["p_w1"], np.float32),
        "p_b1": _colchunks(np.asarray(p["p_b1"]), 4),
        "z_w0a": np.asarray(p["z_w0"], np.float32)[:P],
        "z_w0b": np.asarray(p["z_w0"], np.float32)[P:],
        "z_b0": _colchunks(np.asarray(p["z_b0"]), 8),
        "z_w1": np.asarray(p["z_w1"], np.float32),
        "z_b1": _colchunks(np.asarray(p["z_b1"]), 8),
        "t_posT": _pos_feat_T(),
    }
    z_w2 = np.asarray(p["z_w2"], np.float32)
    p_w2 = np.asarray(p["p_w2"], np.float32)
    p_b2 = np.asarray(p["p_b2"], np.float32)
    z_b2 = np.asarray(p["z_b2"], np.float32)
    in_maps = []
    for c in range(NCORES):
        m = dict(base)
        m["z_w2s"] = np.ascontiguousarray(z_w2[:, c * JLOC:(c + 1) * JLOC])
        m["p_w2s"] = np.ascontiguousarray(p_w2[:, c * FLOC:(c + 1) * FLOC])
        m["p_b2s"] = np.ascontiguousarray(
            p_b2[c * FLOC:(c + 1) * FLOC].reshape(FLOC, 1))
        zb2s = z_b2[c * JLOC:(c + 1) * JLOC].reshape(FLOC, STATE)
        m["z_b2sT"] = np.ascontiguousarray(np.pad(zb2s, ((0, P - FLOC), (0, 0))))
        in_maps.append(m)
    return in_maps


def kernel(state, action, xt_init, noise, params, _trace=False):
    if "nc" not in _CACHE:
        _CACHE["nc"] = build(T)
    nc = _CACHE["nc"]
    in_maps = _prep_inputs(state, action, xt_init, noise, params)
    r = run_bass_kernel_spmd(nc, in_maps, list(range(NCORES)), trace=_trace)
    _CACHE["last_result"] = r
    out = r.results[0]["out_xtT"]
    return np.ascontiguousarray(out.T.astype(np.float32))


# revision 17
# speedup vs baseline: 1.0463x; 1.0065x over previous
"""DDPM sampling kernel for Trainium2 (8 NeuronCores, Bass/Tile).

Strategy:
  - Factorized psi(state, action) computed once on-device (replicated).
  - 50 sequential reverse-diffusion steps, fully unrolled.
  - zeta MLP feature-major on-chip: L0 (128->1024), L1 (1024->1024),
    L2 (1024->32768) with the 32768-wide output projection column-sharded
    8 ways (32 FEAT-features per core, z_w2 shard SBUF-resident).
  - score = einsum('bf,bfs->bs', psi, zeta) folded into the L2 psum drain
    via scalar_tensor_tensor with per-partition psi scalars (batch-major).
  - Per-step AllReduce(score partial, 512KB) across the 8 cores.
  - Mish via rational identity mish(x) = x - x * 2/((1+e^x)^2+1)
    (ACT Exp+Square, DVE clamp/reciprocal/fused-mac) - no table switches.
  - fp32r (TF32-class) matmuls everywhere, N>=256 for full PE rate.

Layouts: activations feature-major [feat_part, B_free]; L2 psum batch-major
[b_part, j_free]; xt update feature-major after PE-transposing the
AllReduced score. Host pre-transposes inputs/outputs.
"""
import math
import numpy as np

import concourse.bass as bass
import concourse.mybir as mybir
import concourse.tile as tile
from concourse import bacc
from concourse.bass_utils import run_bass_kernel_spmd
from concourse.masks import make_identity

P = 128
B = 1024
T = 50
STATE = 128
ACT_D = 64
EMBED = 128
FEAT = 256
NCORES = 8
FLOC = FEAT // NCORES          # 32 features per core
JLOC = FLOC * STATE            # 4096 z_w2 columns per core
X_MIN, X_MAX = -1.0, 1.0
BETA_MIN, BETA_MAX = 1e-4, 0.02
MAX_POS = 10000.0

F32 = mybir.dt.float32
F32R = mybir.dt.float32r
AF = mybir.ActivationFunctionType
ALU = mybir.AluOpType

RSQ2 = float(1.0 / math.sqrt(2.0))
CLAMP = 3.0e38

_CACHE = {}


def _schedule():
    betas = np.linspace(BETA_MIN, BETA_MAX, T).astype(np.float64)
    alphas = 1.0 - betas
    abars = np.cumprod(alphas)
    abars_prev = np.concatenate([[1.0], abars[:-1]])
    # step i uses t = T-1-i
    c0c1 = []
    c1 = []
    sig = []
    for i in range(T):
        t = T - 1 - i
        beta = betas[t]
        ab = abars[t]
        abp = abars_prev[t]
        _c1 = 1.0 / math.sqrt(alphas[t])
        _c0 = beta / math.sqrt(1.0 - ab)
        s2 = max(beta * (1.0 - abp) / (1.0 - ab), 1e-20)
        _s = math.sqrt(s2) if t > 0 else 0.0
        c0c1.append(np.float32(_c0 * _c1))
        c1.append(np.float32(_c1))
        sig.append(np.float32(_s))
    return c0c1, c1, sig


def _pos_feat_T():
    # pos_feat for t-values in step order i -> t = T-1-i; returns [128, T]
    half = EMBED // 2
    freqs = (np.arange(half, dtype=np.float64) / half)
    freqs = (1.0 / MAX_POS) ** freqs
    ts = np.array([T - 1 - i for i in range(T)], dtype=np.float64)[:, None]
    x = ts @ freqs[None, :]                      # [T, half]
    pf = np.concatenate([np.cos(x), np.sin(x)], axis=1)   # [T, 128]
    return np.ascontiguousarray(pf.T.astype(np.float32))  # [128, T]


def _colchunks(v, nch):
    # bias vector [nch*128] -> [128, nch] (column k = bias[128k:128k+128])
    return np.ascontiguousarray(v.reshape(nch, P).T.astype(np.float32))


def _mish_drain(nc, sb, psum, bias_col, out_f32r_ap, wq, tag, rsq2=None):
    """mish(psum + bias) -> out (F32R). psum: [128, N] PSUM AP.
    wq: scratch pool (2 tags; r reuses w, u reuses q)."""
    n = psum.shape[-1]
    w = wq.tile([P, n], F32, tag=f"w{tag}", name=f"w_{tag}_{nc.next_id()}")
    # w = exp(x);  q' = ((w+1)/sqrt2)^2 = ((w+1)^2)/2
    nc.scalar.activation(w[:], psum, AF.Exp, bias=bias_col)
    q = wq.tile([P, n], F32, tag=f"q{tag}", name=f"q_{tag}_{nc.next_id()}")
    nc.scalar.activation(q[:], w[:], AF.Square, bias=rsq2, scale=rsq2)
    # q2h = clamp(q' + 0.5)  ( = ((1+e^x)^2+1)/2 );  r2 = 1/q2h = 2/(...)
    nc.vector.tensor_scalar(q[:], q[:], 0.5, CLAMP, ALU.add, ALU.min)
    nc.vector.reciprocal_approx_fast(w[:], q[:])          # r2 -> w slot
    # u2 = (psum + bias) * r2 ;  out = (psum + bias) - u2
    nc.vector.scalar_tensor_tensor(q[:], psum, bias_col, w[:], ALU.add, ALU.mult)
    nc.vector.scalar_tensor_tensor(out_f32r_ap, psum, bias_col, q[:],
                                   ALU.add, ALU.subtract)


def build(t_steps=T):
    nc = bacc.Bacc("TRN2", target_bir_lowering=False, debug=True)

    def din(name, shape):
        return nc.dram_tensor(name, list(shape), F32, kind="ExternalInput")

    # ---- inputs (feature-major where consumed as matmul rhs) ----
    stateT = din("stateT", [P, B])
    actionT = din("actionT", [P, B])           # padded rows 64:128 = 0
    xt0T = din("xt0T", [P, B])
    noiseT = din("noiseT", [T, P, B])
    s_w1 = din("s_w1", [P, 2 * EMBED]); s_b1 = din("s_b1", [P, 2])
    s_w2 = din("s_w2", [2 * EMBED, EMBED]); s_b2 = din("s_b2", [P, 1])
    a_w1 = din("a_w1", [P, 2 * EMBED]); a_b1 = din("a_b1", [P, 2])   # padded
    a_w2 = din("a_w2", [2 * EMBED, EMBED]); a_b2 = din("a_b2", [P, 1])
    t_w1 = din("t_w1", [P, 2 * EMBED]); t_b1 = din("t_b1", [P, 2])
    t_w2 = din("t_w2", [2 * EMBED, EMBED]); t_b2 = din("t_b2", [P, 1])
    p_w0 = din("p_w0", [2 * EMBED, 512]); p_b0 = din("p_b0", [P, 4])
    p_w1 = din("p_w1", [512, 512]); p_b1 = din("p_b1", [P, 4])
    p_w2s = din("p_w2s", [512, FLOC]); p_b2s = din("p_b2s", [FLOC, 1])
    z_w0a = din("z_w0a", [P, 1024])            # z_w0[:128, :]
    z_w0b = din("z_w0b", [P, 1024])            # z_w0[128:, :]
    z_b0 = din("z_b0", [P, 8])
    z_w1 = din("z_w1", [1024, 1024])
    z_b1 = din("z_b1", [P, 8])
    z_w2s = din("z_w2s", [1024, JLOC])         # per-core column shard
    z_b2sT = din("z_b2sT", [P, P])             # [f(32,pad128), s(128)]
    t_posT = din("t_posT", [P, T])
    out_xtT = nc.dram_tensor("out_xtT", [P, B], F32, kind="ExternalOutput")

    c0c1, c1v, sigv = _schedule()

    with tile.TileContext(nc) as tc:
        # ---------------- pools ----------------
        wpool = tc.alloc_tile_pool(name="wpool", bufs=1)
        ps = tc.alloc_tile_pool(name="ps", bufs=1, space="PSUM")
        _psctr = [0]

        def pstile(shape, name):
            _psctr[0] += 1
            return ps.tile(shape, F32, tag=f"psA{_psctr[0] % 4}", name=name)
        dram = tc.alloc_tile_pool(name="dram", bufs=2, space="DRAM")

        # resident constants (small); big weights allocated after setup frees
        zb1 = wpool.tile([P, 8], F32)
        nc.sync.dma_start(zb1[:], z_b1[:])
        ident = wpool.tile([P, P], F32)
        make_identity(nc, ident[:])
        c_rsq2 = wpool.tile([P, 1], F32)
        nc.vector.memset(c_rsq2[:], RSQ2)
        psi_bm = wpool.tile([P, 8, FLOC], F32)      # psi batch-major per btile
        sb_dram = dram.tile([P, 8, P], F32, tag="sbd", name="sb_dram")
        b0T = wpool.tile([P, 8, T], F32)            # per-step L0 bias columns

        # ---------------- setup: psi, score-bias, t-embedding ----------------
        with tc.tile_pool(name="setup", bufs=1) as su:
            def ld(dr, shape, dtype=F32R, name=None):
                t_ = su.tile(list(shape), dtype, name=name or dr.name + "_t")
                if dtype == F32R:
                    nc.gpsimd.dma_start(t_[:], dr[:])
                else:
                    nc.sync.dma_start(t_[:], dr[:])
                return t_

            def ldc(dr, nch, f, name=None):
                # chunked weight [nch*128, f] -> [P, nch, f] (k%128 on partitions)
                t_ = su.tile([P, nch, f], F32R, name=name or dr.name + "_t")
                nc.gpsimd.dma_start(
                    t_[:], dr.ap().rearrange("(ko p) f -> p ko f", p=P))
                return t_

            sw1 = ld(s_w1, [P, 256]); aw1 = ld(a_w1, [P, 256])
            sw2 = ldc(s_w2, 2, P); aw2 = ldc(a_w2, 2, P)
            pw0 = ldc(p_w0, 2, 512); pw1 = ldc(p_w1, 4, 512)
            pw2 = ldc(p_w2s, 4, FLOC)
            tw1 = ld(t_w1, [P, 256]); tw2 = ldc(t_w2, 2, P)
            sb1 = ld(s_b1, [P, 2], F32); sb2 = ld(s_b2, [P, 1], F32)
            ab1 = ld(a_b1, [P, 2], F32); ab2 = ld(a_b2, [P, 1], F32)
            tb1 = ld(t_b1, [P, 2], F32); tb2 = ld(t_b2, [P, 1], F32)
            pb0 = ld(p_b0, [P, 4], F32); pb1 = ld(p_b1, [P, 4], F32)
            pb2 = ld(p_b2s, [FLOC, 1], F32)
            zb0 = ld(z_b0, [P, 8], F32)
            zw0b = ld(z_w0b, [P, 1024])
            zb2t = ld(z_b2sT, [P, P])
            stT = ld(stateT, [P, B]); acT = ld(actionT, [P, B])
            tpos = ld(t_posT, [P, T])

            # -- t embedding chain -> b0T --
            thm = su.tile([P, 2, T], F32R, name="thm")
            for fc in range(2):
                pt = pstile([P, T], f"su_th_{fc}")
                nc.tensor.matmul(pt[:], tw1[:, bass.ts(fc, P)], tpos[:],
                                 start=True, stop=True)
                _mish_drain(nc, su, pt[:], tb1[:, fc:fc + 1],
                            thm[:, fc, :], su, "su", c_rsq2[:])
            tff = su.tile([P, T], F32R, name="tff")
            pt = pstile([P, T], "su_tff")
            for k in range(2):
                nc.tensor.matmul(pt[:], tw2[:, k, :], thm[:, k, :],
                                 start=(k == 0), stop=(k == 1))
            nc.scalar.activation(tff[:], pt[:], AF.Identity, bias=tb2[:, 0:1])
            for fc in range(8):
                pt = pstile([P, T], f"su_b0_{fc}")
                nc.tensor.matmul(pt[:], zw0b[:, bass.ts(fc, P)], tff[:],
                                 start=True, stop=True)
                nc.scalar.activation(b0T[:, fc, :], pt[:], AF.Identity,
                                     bias=zb0[:, fc:fc + 1])

            # -- psi chain (feature-major, full B) --
            shm = su.tile([P, 2, B], F32R, name="shm")
            ahm = su.tile([P, 2, B], F32R, name="ahm")
            for fc in range(2):
                for h in range(2):
                    bsl = bass.ts(h, 512)
                    pt = pstile([P, 512], f"su_sh_{fc}_{h}")
                    nc.tensor.matmul(pt[:], sw1[:, bass.ts(fc, P)], stT[:, bsl],
                                     start=True, stop=True)
                    _mish_drain(nc, su, pt[:], sb1[:, fc:fc + 1],
                                shm[:, fc, bsl], su, "su", c_rsq2[:])
                    pt = pstile([P, 512], f"su_ah_{fc}_{h}")
                    nc.tensor.matmul(pt[:], aw1[:, bass.ts(fc, P)], acT[:, bsl],
                                     start=True, stop=True)
                    _mish_drain(nc, su, pt[:], ab1[:, fc:fc + 1],
                                ahm[:, fc, bsl], su, "su", c_rsq2[:])
            xp = su.tile([P, 2, B], F32R, name="xp")   # [s_ff ; a_ff]
            for h in range(2):
                bsl = bass.ts(h, 512)
                pt = pstile([P, 512], f"su_sff_{h}")
                for k in range(2):
                    nc.tensor.matmul(pt[:], sw2[:, k, :], shm[:, k, bsl],
                                     start=(k == 0), stop=(k == 1))
                nc.scalar.activation(xp[:, 0, bsl], pt[:], AF.Identity, bias=sb2[:, 0:1])
                pt = pstile([P, 512], f"su_aff_{h}")
                for k in range(2):
                    nc.tensor.matmul(pt[:], aw2[:, k, :], ahm[:, k, bsl],
                                     start=(k == 0), stop=(k == 1))
                nc.scalar.activation(xp[:, 1, bsl], pt[:], AF.Identity, bias=ab2[:, 0:1])
            phm = su.tile([P, 4, B], F32R, name="phm")
            for fc in range(4):
                for h in range(2):
                    bsl = bass.ts(h, 512)
                    pt = pstile([P, 512], f"su_ph_{fc}_{h}")
                    for k in range(2):
                        nc.tensor.matmul(pt[:], pw0[:, k, bass.ts(fc, P)],
                                         xp[:, k, bsl], start=(k == 0), stop=(k == 1))
                    _mish_drain(nc, su, pt[:], pb0[:, fc:fc + 1],
                                phm[:, fc, bsl], su, "su", c_rsq2[:])
            ph1m = su.tile([P, 4, B], F32R, name="ph1m")
            for fc in range(4):
                for h in range(2):
                    bsl = bass.ts(h, 512)
                    pt = pstile([P, 512], f"su_ph1_{fc}_{h}")
                    for k in range(4):
                        nc.tensor.matmul(pt[:], pw1[:, k, bass.ts(fc, P)],
                                         phm[:, k, bsl], start=(k == 0), stop=(k == 3))
                    _mish_drain(nc, su, pt[:], pb1[:, fc:fc + 1],
                                ph1m[:, fc, bsl], su, "su", c_rsq2[:])
            # psi_loc [32(pad128), B] fp32, padded rows zero
            psiT = su.tile([P, B], F32, name="psiT")
            nc.vector.memset(psiT[:], 0.0)
            for h in range(2):
                bsl = bass.ts(h, 512)
                pt = pstile([FLOC, 512], f"su_psi_{h}")
                for k in range(4):
                    nc.tensor.matmul(pt[:], pw2[:, k, :], ph1m[:, k, bsl],
                                     start=(k == 0), stop=(k == 3))
                nc.scalar.activation(psiT[:FLOC, bsl], pt[:], AF.Identity,
                                     bias=pb2[:, 0:1])
            psiTr = su.tile([P, B], F32R, name="psiTr")
            sb_su = su.tile([P, 8, P], F32, name="sb_su")
            nc.gpsimd.tensor_copy(psiTr[:], psiT[:])
            # psi_bm via PE transpose; sb_m = psi_loc.T @ z_b2sT
            for m in range(8):
                msl = bass.ts(m, P)
                pt = pstile([P, P], f"su_tr_{m}")
                nc.tensor.transpose(pt[:], psiT[:, msl], ident[:])
                nc.scalar.activation(psi_bm[:, m, :], pt[:, :FLOC], AF.Copy)
                pt2 = pstile([P, P], f"su_sb_{m}")
                nc.tensor.matmul(pt2[:], psiTr[:, msl], zb2t[:],
                                 start=True, stop=True)
                nc.scalar.activation(sb_su[:, m, :], pt2[:], AF.Copy)

            nc.sync.dma_start(sb_dram[:], sb_su[:])

        # ---------------- resident weights (allocated post-setup) ----------
        wbig = tc.alloc_tile_pool(name="wbig", bufs=1)
        zw2 = wbig.tile([P, 8, JLOC], F32R)        # 128KB/part
        nc.gpsimd.dma_start(zw2[:], z_w2s.ap().rearrange("(ko p) j -> p ko j", p=P))
        zw0a = wbig.tile([P, 1024], F32R)
        nc.gpsimd.dma_start(zw0a[:], z_w0a[:])
        zw1r = z_w1.ap().rearrange("(ko p) f -> p ko f", p=P)

        # ---------------- main loop pools ----------------
        h0p = tc.alloc_tile_pool(name="h0p", bufs=1)
        h1p = tc.alloc_tile_pool(name="h1p", bufs=1)
        scr = tc.alloc_tile_pool(name="scr", bufs=1)
        xtp = tc.alloc_tile_pool(name="xtp", bufs=2)
        nzp = tc.alloc_tile_pool(name="nzp", bufs=1)
        scp = tc.alloc_tile_pool(name="scp", bufs=1)
        scorep = tc.alloc_tile_pool(name="scorep", bufs=1)
        zw1p = tc.alloc_tile_pool(name="zw1p", bufs=2)

        xtT = xtp.tile([P, B], F32R, name="xt_init")
        nc.gpsimd.dma_start(xtT[:], xt0T[:])

        NQ = 2
        QB = B // NQ   # 512

        for i in range(t_steps):
            nzT = nzp.tile([P, B], F32, tag="nz", name=f"nz_{i}")
            nc.sync.dma_start(nzT[:], noiseT[i])
            score_m = scorep.tile([P, 8, P], F32, tag="score", name=f"score_{i}")
            nc.sync.dma_start(score_m[:], sb_dram[:])
            xt_new = xtp.tile([P, B], F32R, tag="xt", name=f"xt_{i}")

            h0m_q = []
            h1m_q = []
            for q in range(NQ):
                qsl = bass.ts(q, QB)
                # ---- L0: h0 = mish(xt @ z_w0a + b0_t) ----
                h0m = h0p.tile([P, 8, QB], F32R, tag="h0", name=f"h0_{i}_{q}")
                h0m_q.append(h0m)
                for fc in range(8):
                    pt = ps.tile([P, QB], F32, tag=f"ps{'AB'[fc // 4]}{fc % 4}",
                                 name=f"l0_{i}_{q}_{fc}")
                    nc.tensor.matmul(pt[:], zw0a[:, bass.ts(fc, P)], xtT[:, qsl],
                                     start=True, stop=True)
                    _mish_drain(nc, scr, pt[:],
                                b0T[:, fc, i:i + 1], h0m[:, fc, :], scr, "m", c_rsq2[:])
                # ---- L1 (k-outer, z_w1 streamed, all 8 psum banks) ----
                h1m = h1p.tile([P, 8, QB], F32R, tag="h1", name=f"h1_{i}_{q}")
                h1m_q.append(h1m)
                l1ps = [ps.tile([P, QB], F32,
                                tag=f"ps{'AB'[fc // 4]}{fc % 4}",
                                name=f"l1_{i}_{q}_{fc}") for fc in range(8)]
                for k in range(8):
                    zw1t = zw1p.tile([P, 1024], F32R, tag="zw1",
                                     name=f"zw1_{i}_{q}_{k}")
                    nc.gpsimd.dma_start(zw1t[:], zw1r[:, k, :])
                    for fc in range(8):
                        nc.tensor.matmul(l1ps[fc][:], zw1t[:, bass.ts(fc, P)],
                                         h0m[:, k, :], start=(k == 0), stop=(k == 7))
                for fc in range(8):
                    _mish_drain(nc, scr, l1ps[fc][:],
                                zb1[:, fc:fc + 1], h1m[:, fc, :], scr, "m", c_rsq2[:])
                # ---- L2 + score ----
                for mb in range(4):
                    m = q * 4 + mb
                    for jt in range(8):
                        pt = ps.tile([P, 512], F32, tag=f"ps{'AB'[jt // 4]}{jt % 4}",
                                     name=f"l2_{i}_{m}_{jt}")
                        for k in range(8):
                            nc.tensor.matmul(
                                pt[:], h1m[:, k, bass.ts(mb, P)],
                                zw2[:, k, bass.ts(jt, 512)],
                                start=(k == 0), stop=(k == 7))
                        for sub in range(4):
                            f = 4 * jt + sub
                            nc.vector.scalar_tensor_tensor(
                                score_m[:, m, :], pt[:, bass.ts(sub, P)],
                                psi_bm[:, m, f:f + 1], score_m[:, m, :],
                                ALU.mult, ALU.add)

                if True:
                    # ---- half complete: AllReduce + update this half ----
                    hh = q
                    hsl = bass.ds(hh * 512, 512)
                    arin = dram.tile([P, 4, P], F32, tag=f"arin{hh}",
                                     name=f"arin_{i}_{hh}")
                    arout = dram.tile([P, 4, P], F32, tag=f"arout{hh}",
                                      name=f"arout_{i}_{hh}")
                    nc.sync.dma_start(arin[:], score_m[:, 4 * hh:4 * hh + 4, :])
                    nc.gpsimd.collective_compute(
                        "AllReduce", ALU.add,
                        replica_groups=[list(range(NCORES))],
                        ins=[arin[:].opt()], outs=[arout[:].opt()])
                    ar_rd = scp.tile([P, 4, P], F32, tag=f"arrd{hh}",
                                     name=f"arrd_{i}_{hh}")
                    nc.sync.dma_start(ar_rd[:], arout[:])
                    # update in-place over ar_rd: after each transpose consumes
                    # its slice, the slice becomes the update accumulator.
                    upd = ar_rd
                    for mm_ in range(4):
                        m_g = 4 * hh + mm_
                        pt2 = ps.tile([P, P], F32,
                                      tag=f"ps{'AB'[m_g // 4]}{m_g % 4}",
                                      name=f"tr_{i}_{m_g}")
                        nc.tensor.transpose(pt2[:], ar_rd[:, mm_, :], ident[:])
                        bsl2 = bass.ds(hh * 512 + mm_ * P, P)
                        nc.vector.tensor_scalar_mul(
                            upd[:, mm_, :], xtT[:, bsl2].bitcast(F32),
                            float(c1v[i]))
                        nc.vector.scalar_tensor_tensor(
                            upd[:, mm_, :], pt2[:], -float(c0c1[i]),
                            upd[:, mm_, :], ALU.mult, ALU.add)
                    uv = upd[:].rearrange("p a b -> p (a b)")
                    nc.vector.scalar_tensor_tensor(
                        uv, nzT[:, hsl], float(sigv[i]), uv,
                        ALU.mult, ALU.add)
                    nc.vector.tensor_scalar(xt_new[:, hsl], uv,
                                            X_MIN, X_MAX, ALU.max, ALU.min)

            xtT = xt_new

        nc.sync.dma_start(out_xtT[:], xtT[:].bitcast(F32))

        for p_ in (zw1p, scorep, scp, nzp, xtp, scr, h1p, h0p, dram, ps, wbig, wpool):
            p_.release()

    nc.compile()
    return nc


def _prep_inputs(state, action, xt_init, noise, params):
    p = params
    base = {
        "stateT": np.ascontiguousarray(state.T.astype(np.float32)),
        "actionT": np.ascontiguousarray(
            np.pad(action.T.astype(np.float32), ((0, P - ACT_D), (0, 0)))),
        "xt0T": np.ascontiguousarray(xt_init.T.astype(np.float32)),
        "noiseT": np.ascontiguousarray(noise.transpose(0, 2, 1).astype(np.float32)),
        "s_w1": np.asarray(p["s_w1"], np.float32), "s_b1": _colchunks(np.asarray(p["s_b1"]), 2),
        "s_w2": np.asarray(p["s_w2"], np.float32),
        "s_b2": _colchunks(np.asarray(p["s_b2"]), 1),
        "a_w1": np.pad(np.asarray(p["a_w1"], np.float32), ((0, P - ACT_D), (0, 0))),
        "a_b1": _colchunks(np.asarray(p["a_b1"]), 2),
        "a_w2": np.asarray(p["a_w2"], np.float32),
        "a_b2": _colchunks(np.asarray(p["a_b2"]), 1),
        "t_w1": np.asarray(p["t_w1"], np.float32), "t_b1": _colchunks(np.asarray(p["t_b1"]), 2),
        "t_w2": np.asarray(p["t_w2"], np.float32),
        "t_b2": _colchunks(np.asarray(p["t_b2"]), 1),
        "p_w0": np.asarray(p["p_w0"], np.float32),
        "p_b0": _colchunks(np.asarray(p["p_b0"]), 4),
        "p_w1": np.asarray(p["p_w1"], np.float32),
        "p_b1": _colchunks(np.asarray(p["p_b1"]), 4),
        "z_w0a": np.asarray(p["z_w0"], np.float32)[:P],
        "z_w0b": np.asarray(p["z_w0"], np.float32)[P:],
        "z_b0": _colchunks(np.asarray(p["z_b0"]), 8),
        "z_w1": np.asarray(p["z_w1"], np.float32),
        "z_b1": _colchunks(np.asarray(p["z_b1"]), 8),
        "t_posT": _pos_feat_T(),
    }
    z_w2 = np.asarray(p["z_w2"], np.float32)
    p_w2 = np.asarray(p["p_w2"], np.float32)
    p_b2 = np.asarray(p["p_b2"], np.float32)
    z_b2 = np.asarray(p["z_b2"], np.float32)
    in_maps = []
    for c in range(NCORES):
        m = dict(base)
        m["z_w2s"] = np.ascontiguousarray(z_w2[:, c * JLOC:(c + 1) * JLOC])
        m["p_w2s"] = np.ascontiguousarray(p_w2[:, c * FLOC:(c + 1) * FLOC])
        m["p_b2s"] = np.ascontiguousarray(
            p_b2[c * FLOC:(c + 1) * FLOC].reshape(FLOC, 1))
        zb2s = z_b2[c * JLOC:(c + 1) * JLOC].reshape(FLOC, STATE)
        m["z_b2sT"] = np.ascontiguousarray(np.pad(zb2s, ((0, P - FLOC), (0, 0))))
        in_maps.append(m)
    return in_maps


def kernel(state, action, xt_init, noise, params, _trace=False):
    if "nc" not in _CACHE:
        _CACHE["nc"] = build(T)
    nc = _CACHE["nc"]
    in_maps = _prep_inputs(state, action, xt_init, noise, params)
    r = run_bass_kernel_spmd(nc, in_maps, list(range(NCORES)), trace=_trace)
    _CACHE["last_result"] = r
    out = r.results[0]["out_xtT"]
    return np.ascontiguousarray(out.T.astype(np.float32))
